# revision 23
# baseline (speedup 1.0000x reference)
"""DCNv2 deformable PS-RoI pooling on 8 Trainium2 NeuronCores (v2).

Strategy (RoI-data-parallel, 32 rois per core):
  * Host replicates the reference coordinate math exactly in float32.
    Bilinear weights / validity / 1-over-count factor per-bin separably:
    A[(y,x), j] = Wy[y, j] * Wx[x, j], so only pixels with
    (Wy row nonzero) x (Wx col nonzero) are needed -- the touched set is
    exactly a cartesian product ys x xs (~55% of the padded bbox).
  * Touched pixels of the 32 rois are bin-packed densely into 128-row
    chunks (per psum-group of rois), removing per-roi 128-padding.
  * Device (SPMD): indirect-DMA gather of pixel channel vectors
    (512B each, full DMA efficiency), matmul patch^T @ A per chunk into
    per-group PSUM banks, PSUM->SBUF copies split across DVE/Act,
    per-group output DMAs. All DMACopies dispatch from the Pool queue
    (cheap 25ns dispatch).
"""
import numpy as np

f32 = np.float32
f64 = np.float64

B, C, H, W = 8, 256, 64, 64
N_ROIS, P, S = 256, 7, 4
NJ = P * P  # 49
SCALE = f32(1.0 / 16.0)
TRANS_STD = f32(0.1)
N_CORES = 8
RPC = N_ROIS // N_CORES  # 32 rois per core
GROUP_SIZES = (4,) * 8  # 8 psum groups of 4 slots; banks rotate (bufs=4)
# processing order: smallish group first (PE starts early), big mixed
# middle (rems pack well), smallest group last (short drain tail)
_DEAL = (0, 1, 2, 3, 4, 5, 5, 4, 3, 2, 1, 0)
GROUP_RANKS = (
    tuple(range(24, 28)),
    *(tuple(r for r in range(24) if _DEAL[r % 12] == d) for d in range(6)),
    tuple(range(28, 32)),
)
# gather call boundaries at ends of these groups (5 calls)
GCALL_END_GROUPS = (0, 2, 4, 6, 7)
N_GCALLS = len(GCALL_END_GROUPS)

_prog_cache = {}


# --------------------------------------------------------------------------
# host math: exact f32 replication, separable per-bin weights
# --------------------------------------------------------------------------
def _sep_weights(rois, offset):
    """Per roi: (batch, ys, xs, Wy (ny,49) f64, Wx (nx,49) f64)."""
    rois = np.asarray(rois, dtype=f32)
    offset = np.asarray(offset, dtype=f32)
    N = rois.shape[0]
    batch = rois[:, 0].astype(np.int32)

    roi_sw = np.round(rois[:, 1]) * SCALE - f32(0.5)
    roi_sh = np.round(rois[:, 2]) * SCALE - f32(0.5)
    roi_ew = (np.round(rois[:, 3]) + f32(1.0)) * SCALE - f32(0.5)
    roi_eh = (np.round(rois[:, 4]) + f32(1.0)) * SCALE - f32(0.5)
    roi_w = np.maximum(roi_ew - roi_sw, f32(0.1))
    roi_h = np.maximum(roi_eh - roi_sh, f32(0.1))
    bin_w = roi_w / f32(P)
    bin_h = roi_h / f32(P)
    sub_w = bin_w / f32(S)
    sub_h = bin_h / f32(S)

    ph = np.arange(P, dtype=np.int32)
    pw = np.arange(P, dtype=np.int32)
    part_h = np.clip(
        np.floor(ph.astype(f32) / f32(P) * f32(P)).astype(np.int32), 0, P - 1
    )
    part_w = np.clip(
        np.floor(pw.astype(f32) / f32(P) * f32(P)).astype(np.int32), 0, P - 1
    )
    tx = offset[:, 0][:, part_h[:, None], part_w[None, :]] * TRANS_STD  # (N,7,7)
    ty = offset[:, 1][:, part_h[:, None], part_w[None, :]] * TRANS_STD

    wstart = (
        pw.astype(f32)[None, None, :] * bin_w[:, None, None]
        + roi_sw[:, None, None]
        + tx * roi_w[:, None, None]
    )  # (N,7,7)
    hstart = (
        ph.astype(f32)[None, :, None] * bin_h[:, None, None]
        + roi_sh[:, None, None]
        + ty * roi_h[:, None, None]
    )

    samp = np.arange(S, dtype=f32)
    wpos = wstart[..., None] + samp * sub_w[:, None, None, None]  # (N,7,7,4)
    hpos = hstart[..., None] + samp * sub_h[:, None, None, None]

    vw = (wpos >= f32(-0.5)) & (wpos <= f32(W) - f32(0.5))
    vh = (hpos >= f32(-0.5)) & (hpos <= f32(H) - f32(0.5))
    wc = np.clip(wpos, f32(0.0), f32(W - 1.0))
    hc = np.clip(hpos, f32(0.0), f32(H - 1.0))

    x0 = np.floor(wc).astype(np.int64)
    x1 = np.ceil(wc).astype(np.int64)
    y0 = np.floor(hc).astype(np.int64)
    y1 = np.ceil(hc).astype(np.int64)
    dx = (wc - np.floor(wc)).astype(f64)
    dy = (hc - np.floor(hc)).astype(f64)

    cnt_h = vh.sum(axis=3)  # (N,7,7)
    cnt_w = vw.sum(axis=3)
    ch = 1.0 / np.maximum(cnt_h, 1).astype(f64)
    cw = 1.0 / np.maximum(cnt_w, 1).astype(f64)

    jidx = (ph[:, None] * P + pw[None, :]).astype(np.int64)  # (7,7)
    jb = np.broadcast_to(jidx[None, :, :, None], (N, P, P, S))
    nb = np.broadcast_to(np.arange(N, dtype=np.int64)[:, None, None, None],
                         (N, P, P, S))

    Wy_full = np.zeros((N, H, NJ), f64)
    why0 = (1.0 - dy) * vh * ch[..., None]
    why1 = dy * vh * ch[..., None]
    np.add.at(Wy_full, (nb, y0, jb), why0)
    np.add.at(Wy_full, (nb, y1, jb), why1)

    Wx_full = np.zeros((N, W, NJ), f64)
    wwx0 = (1.0 - dx) * vw * cw[..., None]
    wwx1 = dx * vw * cw[..., None]
    np.add.at(Wx_full, (nb, x0, jb), wwx0)
    np.add.at(Wx_full, (nb, x1, jb), wwx1)

    out = []
    for n in range(N):
        ys = np.nonzero(np.abs(Wy_full[n]).sum(axis=1) > 0)[0]
        xs = np.nonzero(np.abs(Wx_full[n]).sum(axis=1) > 0)[0]
        out.append((int(batch[n]), ys.astype(np.int32), xs.astype(np.int32),
                    Wy_full[n][ys], Wx_full[n][xs]))
    return out


# --------------------------------------------------------------------------
# packing plan (shared structure across the 8 cores)
# --------------------------------------------------------------------------
def _make_plan(npads):
    """npads: tuple of 32 ints (desc), rank r -> padded pixel count.

    Returns dict with:
      rank2slot: rank -> output slot index o (0..31); slot o in group g
                 at position pos (o = base[g] + pos, pos = order of first
                 appearance in the group's chunk stream)
      chunks: list of [segments] in stream order; segment =
              (rank, row0, row1, pix0, pair)
      gcall_nch: chunks per gather call
      mchains: per group, list (PE-emission order) of
               (pos0, [(chunk, pair0, npair), ...]) accumulation chains
    """
    base = [0]
    for gs in GROUP_SIZES:
        base.append(base[-1] + gs)
    pair = 0
    all_chunks = []   # list of [ (rank, row0, row1, pix0, pair) ]
    group_chunk_rng = []
    rank2slot = {}
    chain_of_rank = {r: [] for r in range(len(npads))}
    for g, ranks in enumerate(GROUP_RANKS):
        c_start = len(all_chunks)
        ranks_sorted = sorted(ranks, key=lambda r: -npads[r])
        ded = []
        rems = []
        for r in ranks_sorted:
            nfull, rem = divmod(npads[r], 128)
            for i in range(nfull):
                ded.append([(r, 0, 128, i * 128)])
            rems.append((r, max(rem, 1) if nfull == 0 else rem, nfull))
        rems = [t for t in rems if t[1] > 0]
        rems.sort(key=lambda t: -t[1])
        bins = []
        for r, rem, nfull in rems:
            for b in bins:
                if b[0] + rem <= 128:
                    b[1].append((r, b[0], b[0] + rem, nfull * 128))
                    b[0] += rem
                    break
            else:
                bins.append([rem, [(r, 0, rem, nfull * 128)]])
        # bins first: single-chunk chains complete as soon as their
        # chunk lands, giving PE early work; dedicated chunks after
        gchunks = [b[1] for b in bins] + ded
        # assign pairs + positions (first appearance order) + chains
        pos_of = {}
        for segs in gchunks:
            ci = len(all_chunks)
            segs2 = []
            for (r, r0, r1, px) in segs:
                if r not in pos_of:
                    pos_of[r] = len(pos_of)
                segs2.append((r, r0, r1, px, pair))
                chain_of_rank[r].append((px, ci, pair))
                pair += 1
            all_chunks.append(segs2)
        for r, pos in pos_of.items():
            rank2slot[r] = base[g] + pos
        group_chunk_rng.append((c_start, len(all_chunks)))

    T = len(all_chunks)
    gc_bounds = [0] + [group_chunk_rng[g][1] for g in GCALL_END_GROUPS]
    gcall_nch = [gc_bounds[i + 1] - gc_bounds[i] for i in range(N_GCALLS)]

    # per-group chains in pos order, then merge runs of single-chunk
    # chains with consecutive (pos, pair) in the same chunk
    mchains = []
    for g, ranks in enumerate(GROUP_RANKS):
        glist = [None] * len(ranks)
        for r in ranks:
            pos = rank2slot[r] - base[g]
            # emission/accumulation order = chunk order (data availability)
            glist[pos] = sorted(chain_of_rank[r], key=lambda t: t[1])
        merged = []
        pos = 0
        while pos < len(glist):
            ch = glist[pos]
            if len(ch) == 1:
                px, ci, pr = ch[0]
                npair = 1
                while (pos + npair < len(glist)
                       and len(glist[pos + npair]) == 1
                       and glist[pos + npair][0][1] == ci
                       and glist[pos + npair][0][2] == pr + npair):
                    npair += 1
                merged.append((pos, [(ci, pr, npair)]))
                pos += npair
            else:
                merged.append((pos, [(ci, pr, 1) for (px, ci, pr) in ch]))
                pos += 1
        # PE emission order: chains whose data lands earliest first
        merged.sort(key=lambda m: max(it[0] for it in m[1]))
        mchains.append(merged)

    return dict(rank2slot=rank2slot, chunks=all_chunks,
                gcall_nch=gcall_nch, pairs=pair, mchains=mchains,
                base=base)


# --------------------------------------------------------------------------
# device program
# --------------------------------------------------------------------------
def _build_program(npads):
    import concourse.bacc as bacc
    import concourse.mybir as mybir
    from concourse.tile import TileContext

    plan = _make_plan(npads)
    T = len(plan["chunks"])
    PAIRS = plan["pairs"]
    gcall_nch = plan["gcall_nch"]
    base = plan["base"]

    nc = bacc.Bacc("TRN2", num_devices=N_CORES)
    dt = mybir.dt
    fcl = nc.dram_tensor("fcl", [B * H * W, C], dt.float16, kind="ExternalInput")
    amat = nc.dram_tensor("amat", [128, PAIRS, NJ], dt.float16,
                          kind="ExternalInput")
    ca = gcall_nch[0]
    pidxa = nc.dram_tensor("pidxa", [128, ca * 8], dt.int16,
                           kind="ExternalInput")
    pidxb = nc.dram_tensor("pidxb", [128, (T - ca) * 8], dt.int16,
                           kind="ExternalInput")
    outd = nc.dram_tensor("out", [128, RPC, 2, NJ], dt.float16,
                          kind="ExternalOutput")

    gc_bounds = [0]
    for n in gcall_nch:
        gc_bounds.append(gc_bounds[-1] + n)

    def gc_of(ci):
        for gc in range(N_GCALLS):
            if ci < gc_bounds[gc + 1]:
                return gc
        raise AssertionError(ci)

    with TileContext(nc) as tc:
        with (
            tc.tile_pool(name="main", bufs=1) as mp,
            tc.tile_pool(name="psum", bufs=4, space="PSUM") as pp,
        ):
            idxa_t = mp.tile([128, ca * 8], dt.int16, tag="idxa")
            idxb_t = mp.tile([128, (T - ca) * 8], dt.int16, tag="idxb")
            a_t = mp.tile([128, PAIRS, NJ], dt.float16, tag="amat")
            ob = mp.tile([128, RPC, 2, NJ], dt.float16, tag="outbuf")
            warm = mp.tile([128, 8], dt.float16, tag="warm")
            nc.sync.dma_start(out=idxa_t[:], in_=pidxa[:])
            nc.sync.dma_start(out=idxb_t[:], in_=pidxb[:])
            nc.sync.dma_start(out=a_t[:], in_=amat[:])
            # preload the Act engine's activation table (1283ns) upfront
            nc.vector.memset(warm[:], 0.0)
            nc.scalar.activation(warm[:], warm[:],
                                 mybir.ActivationFunctionType.Copy)

            patch = []
            for gc in range(N_GCALLS):
                nch = gcall_nch[gc]
                p_t = mp.tile([128, nch, C], dt.float16, tag=f"patch{gc}")
                c0 = gc_bounds[gc]
                idxs = (idxa_t[:, :] if gc == 0
                        else idxb_t[:, (c0 - ca) * 8:(c0 - ca + nch) * 8])
                nc.gpsimd.dma_gather(
                    out_ap=p_t[:],
                    in_ap=fcl[:],
                    idxs_ap=idxs,
                    num_idxs=nch * 128,
                    num_idxs_reg=nch * 128,
                    elem_size=C,
                    single_packet=False,
                )
                patch.append(p_t)

            for g, gs in enumerate(GROUP_SIZES):
                o0 = base[g]
                pbg = [pp.tile([128, gs, NJ], dt.float32, tag=f"pb{h}",
                               name=f"pb{g}_{h}") for h in range(2)]
                for (pos0, items) in plan["mchains"][g]:
                    for h in range(2):
                        for i, (ci, pr, npair) in enumerate(items):
                            gc = gc_of(ci)
                            lc = ci - gc_bounds[gc]
                            nc.tensor.matmul(
                                out=pbg[h][:, pos0:pos0 + npair, :],
                                lhsT=patch[gc][:, lc, h * 128:(h + 1) * 128],
                                rhs=a_t[:, pr:pr + npair, :],
                                start=(i == 0),
                                stop=(i == len(items) - 1),
                            )
                nc.vector.tensor_copy(out=ob[:, o0:o0 + gs, 0, :],
                                      in_=pbg[0][:])
                nc.scalar.activation(ob[:, o0:o0 + gs, 1, :], pbg[1][:],
                                     mybir.ActivationFunctionType.Copy)
                nc.sync.dma_start(out=outd[:, o0:o0 + gs, :, :],
                                  in_=ob[:, o0:o0 + gs, :, :])
    nc.compile()
    return nc, plan


# --------------------------------------------------------------------------
# entry point
# --------------------------------------------------------------------------
def kernel(input, rois, offset):
    from concourse.bass_utils import run_bass_kernel_spmd

    input = np.asarray(input, dtype=f32)
    wts = _sep_weights(rois, offset)
    npix = np.array([len(t[1]) * len(t[2]) for t in wts])

    # octet sharding: sorted desc, octet r = 8 consecutive rois -> one per
    # core, so per-rank max (npad) is tight; balance per-core totals by
    # giving the octet's biggest roi to the least-loaded core
    order = np.argsort(-npix, kind="stable")
    npads = tuple(int(max(npix[order[8 * r]], 1)) for r in range(RPC))
    core_rois = [[None] * RPC for _ in range(N_CORES)]
    totals = np.zeros(N_CORES, np.int64)
    for r in range(RPC):
        members = order[8 * r:8 * r + 8]  # desc size
        dst = np.argsort(totals, kind="stable")  # asc load
        for j, m in enumerate(members):
            core_rois[dst[j]][r] = int(m)
            totals[dst[j]] += npix[m]

    key = npads
    if key not in _prog_cache:
        _prog_cache[key] = _build_program(npads)
    nc, plan = _prog_cache[key]
    T = len(plan["chunks"])
    PAIRS = plan["pairs"]

    fcl = np.ascontiguousarray(
        input.transpose(0, 2, 3, 1).astype(np.float16)
    ).reshape(B * H * W, C)

    in_maps = []
    for k in range(N_CORES):
        logical = np.zeros(T * 128, np.int32)
        a_arr = np.zeros((128, PAIRS, NJ), np.float16)
        for ci, segs in enumerate(plan["chunks"]):
            for (r, r0, r1, px0, pr) in segs:
                roi = core_rois[k][r]
                b, ys, xs, Wy, Wx = wts[roi]
                ny, nx = len(ys), len(xs)
                np_r = ny * nx
                n = r1 - r0
                pix = px0 + np.arange(n)
                m = pix < np_r
                if not m.any():
                    continue
                pixm = pix[m]
                yy = ys[pixm // nx]
                xx = xs[pixm % nx]
                logical[ci * 128 + r0:ci * 128 + r1][m] = (
                    b * (H * W) + yy.astype(np.int32) * W + xx
                )
                a_arr[np.arange(r0, r1)[m], pr, :] = (
                    Wy[pixm // nx] * Wx[pixm % nx]
                ).astype(np.float16)
        idx16 = np.tile(logical.astype(np.int16).reshape(-1, 16).T, (8, 1))
        ca = plan["gcall_nch"][0]
        in_maps.append({"fcl": fcl, "amat": a_arr,
                        "pidxa": np.ascontiguousarray(idx16[:, :ca * 8]),
                        "pidxb": np.ascontiguousarray(idx16[:, ca * 8:])})

    res = run_bass_kernel_spmd(nc, in_maps, core_ids=list(range(N_CORES)))

    out_full = np.empty((N_ROIS, C, P, P), f32)
    for k in range(N_CORES):
        arr = res.results[k]["out"].astype(f32)  # (128, RPC, 2, 49)
        t = arr.transpose(1, 2, 0, 3).reshape(RPC, C, P, P)
        for r in range(RPC):
            out_full[core_rois[k][r]] = t[plan["rank2slot"][r]]
    return out_full


# revision 28
# speedup vs baseline: 1.0926x; 1.0926x over previous
"""DCNv2 deformable PS-RoI pooling on 8 Trainium2 NeuronCores (v2).

Strategy (RoI-data-parallel, 32 rois per core):
  * Host replicates the reference coordinate math exactly in float32.
    Bilinear weights / validity / 1-over-count factor per-bin separably:
    A[(y,x), j] = Wy[y, j] * Wx[x, j], so only pixels with
    (Wy row nonzero) x (Wx col nonzero) are needed -- the touched set is
    exactly a cartesian product ys x xs (~55% of the padded bbox).
  * Touched pixels of the 32 rois are bin-packed densely into 128-row
    chunks (per psum-group of rois), removing per-roi 128-padding.
  * Device (SPMD): indirect-DMA gather of pixel channel vectors
    (512B each, full DMA efficiency), matmul patch^T @ A per chunk into
    per-group PSUM banks, PSUM->SBUF copies split across DVE/Act,
    per-group output DMAs. All DMACopies dispatch from the Pool queue
    (cheap 25ns dispatch).
"""
import numpy as np

f32 = np.float32
f64 = np.float64

B, C, H, W = 8, 256, 64, 64
N_ROIS, P, S = 256, 7, 4
NJ = P * P  # 49
SCALE = f32(1.0 / 16.0)
TRANS_STD = f32(0.1)
N_CORES = 8
RPC = N_ROIS // N_CORES  # 32 rois per core
GROUP_SIZES = (4,) * 8  # 8 psum groups of 4 slots; banks rotate (bufs=4)
# processing order: smallish group first (PE starts early), big mixed
# middle (rems pack well), smallest group last (short drain tail)
_DEAL = (0, 1, 2, 3, 4, 5, 5, 4, 3, 2, 1, 0)
GROUP_RANKS = (
    tuple(range(24, 28)),
    *(tuple(r for r in range(24) if _DEAL[r % 12] == d) for d in range(6)),
    tuple(range(28, 32)),
)
# gather call boundaries at ends of these groups (6 calls -- last three
# groups land staggered so their copy/out chains overlap)
GCALL_END_GROUPS = (0, 2, 4, 5, 6, 7)
N_GCALLS = len(GCALL_END_GROUPS)
# output DMAs merged over consecutive group pairs
OUT_GROUPS = ((0, 1), (2, 3), (4, 5), (6, 7))

_prog_cache = {}


# --------------------------------------------------------------------------
# host math: exact f32 replication, separable per-bin weights
# --------------------------------------------------------------------------
def _sep_weights(rois, offset):
    """Per roi: (batch, ys, xs, Wy (ny,49) f64, Wx (nx,49) f64)."""
    rois = np.asarray(rois, dtype=f32)
    offset = np.asarray(offset, dtype=f32)
    N = rois.shape[0]
    batch = rois[:, 0].astype(np.int32)

    roi_sw = np.round(rois[:, 1]) * SCALE - f32(0.5)
    roi_sh = np.round(rois[:, 2]) * SCALE - f32(0.5)
    roi_ew = (np.round(rois[:, 3]) + f32(1.0)) * SCALE - f32(0.5)
    roi_eh = (np.round(rois[:, 4]) + f32(1.0)) * SCALE - f32(0.5)
    roi_w = np.maximum(roi_ew - roi_sw, f32(0.1))
    roi_h = np.maximum(roi_eh - roi_sh, f32(0.1))
    bin_w = roi_w / f32(P)
    bin_h = roi_h / f32(P)
    sub_w = bin_w / f32(S)
    sub_h = bin_h / f32(S)

    ph = np.arange(P, dtype=np.int32)
    pw = np.arange(P, dtype=np.int32)
    part_h = np.clip(
        np.floor(ph.astype(f32) / f32(P) * f32(P)).astype(np.int32), 0, P - 1
    )
    part_w = np.clip(
        np.floor(pw.astype(f32) / f32(P) * f32(P)).astype(np.int32), 0, P - 1
    )
    tx = offset[:, 0][:, part_h[:, None], part_w[None, :]] * TRANS_STD  # (N,7,7)
    ty = offset[:, 1][:, part_h[:, None], part_w[None, :]] * TRANS_STD

    wstart = (
        pw.astype(f32)[None, None, :] * bin_w[:, None, None]
        + roi_sw[:, None, None]
        + tx * roi_w[:, None, None]
    )  # (N,7,7)
    hstart = (
        ph.astype(f32)[None, :, None] * bin_h[:, None, None]
        + roi_sh[:, None, None]
        + ty * roi_h[:, None, None]
    )

    samp = np.arange(S, dtype=f32)
    wpos = wstart[..., None] + samp * sub_w[:, None, None, None]  # (N,7,7,4)
    hpos = hstart[..., None] + samp * sub_h[:, None, None, None]

    vw = (wpos >= f32(-0.5)) & (wpos <= f32(W) - f32(0.5))
    vh = (hpos >= f32(-0.5)) & (hpos <= f32(H) - f32(0.5))
    wc = np.clip(wpos, f32(0.0), f32(W - 1.0))
    hc = np.clip(hpos, f32(0.0), f32(H - 1.0))

    x0 = np.floor(wc).astype(np.int64)
    x1 = np.ceil(wc).astype(np.int64)
    y0 = np.floor(hc).astype(np.int64)
    y1 = np.ceil(hc).astype(np.int64)
    dx = (wc - np.floor(wc)).astype(f64)
    dy = (hc - np.floor(hc)).astype(f64)

    cnt_h = vh.sum(axis=3)  # (N,7,7)
    cnt_w = vw.sum(axis=3)
    ch = 1.0 / np.maximum(cnt_h, 1).astype(f64)
    cw = 1.0 / np.maximum(cnt_w, 1).astype(f64)

    jidx = (ph[:, None] * P + pw[None, :]).astype(np.int64)  # (7,7)
    jb = np.broadcast_to(jidx[None, :, :, None], (N, P, P, S))
    nb = np.broadcast_to(np.arange(N, dtype=np.int64)[:, None, None, None],
                         (N, P, P, S))

    Wy_full = np.zeros((N, H, NJ), f64)
    why0 = (1.0 - dy) * vh * ch[..., None]
    why1 = dy * vh * ch[..., None]
    np.add.at(Wy_full, (nb, y0, jb), why0)
    np.add.at(Wy_full, (nb, y1, jb), why1)

    Wx_full = np.zeros((N, W, NJ), f64)
    wwx0 = (1.0 - dx) * vw * cw[..., None]
    wwx1 = dx * vw * cw[..., None]
    np.add.at(Wx_full, (nb, x0, jb), wwx0)
    np.add.at(Wx_full, (nb, x1, jb), wwx1)

    out = []
    for n in range(N):
        ys = np.nonzero(np.abs(Wy_full[n]).sum(axis=1) > 0)[0]
        xs = np.nonzero(np.abs(Wx_full[n]).sum(axis=1) > 0)[0]
        out.append((int(batch[n]), ys.astype(np.int32), xs.astype(np.int32),
                    Wy_full[n][ys], Wx_full[n][xs]))
    return out


# --------------------------------------------------------------------------
# packing plan (shared structure across the 8 cores)
# --------------------------------------------------------------------------
def _make_plan(npads):
    """npads: tuple of 32 ints (desc), rank r -> padded pixel count.

    Returns dict with:
      rank2slot: rank -> output slot index o (0..31); slot o in group g
                 at position pos (o = base[g] + pos, pos = order of first
                 appearance in the group's chunk stream)
      chunks: list of [segments] in stream order; segment =
              (rank, row0, row1, pix0, pair)
      gcall_nch: chunks per gather call
      mchains: per group, list (PE-emission order) of
               (pos0, [(chunk, pair0, npair), ...]) accumulation chains
    """
    base = [0]
    for gs in GROUP_SIZES:
        base.append(base[-1] + gs)
    all_chunks = []       # list of [ (rank, row0, row1, pix0) ], mutable
    group_end = []
    open_bins = []        # [used, seg_list_ref] -- spill across groups
    group_of_rank = {}
    for g, ranks in enumerate(GROUP_RANKS):
        ranks_sorted = sorted(ranks, key=lambda r: -npads[r])
        rems = []
        for r in ranks_sorted:
            group_of_rank[r] = g
            nfull, rem = divmod(npads[r], 128)
            for i in range(nfull):
                all_chunks.append([(r, 0, 128, i * 128)])
            rems.append((r, max(rem, 1) if nfull == 0 else rem, nfull))
        rems = [t for t in rems if t[1] > 0]
        rems.sort(key=lambda t: -t[1])
        for r, rem, nfull in rems:
            for b in open_bins:
                if b[0] + rem <= 128:
                    b[1].append((r, b[0], b[0] + rem, nfull * 128))
                    b[0] += rem
                    break
            else:
                segs = [(r, 0, rem, nfull * 128)]
                all_chunks.append(segs)
                open_bins.append([rem, segs])
        group_end.append(len(all_chunks))

    T = len(all_chunks)
    gc_bounds = [0]
    for g in GCALL_END_GROUPS:
        if group_end[g] > gc_bounds[-1]:
            gc_bounds.append(group_end[g])
    gcall_nch = [gc_bounds[i + 1] - gc_bounds[i]
                 for i in range(len(gc_bounds) - 1)]

    # final pass: assign pair ids in chunk order, positions by first
    # appearance within each group, collect chains
    pair = 0
    rank2slot = {}
    pos_count = [0] * len(GROUP_SIZES)
    chain_of_rank = {r: [] for r in range(len(npads))}
    final_chunks = []
    for ci, segs in enumerate(all_chunks):
        segs2 = []
        for (r, r0, r1, px) in segs:
            if r not in rank2slot:
                g = group_of_rank[r]
                rank2slot[r] = base[g] + pos_count[g]
                pos_count[g] += 1
            segs2.append((r, r0, r1, px, pair))
            chain_of_rank[r].append((px, ci, pair))
            pair += 1
        final_chunks.append(segs2)
    all_chunks = final_chunks

    # per-group chains in pos order, then merge runs of single-chunk
    # chains with consecutive (pos, pair) in the same chunk
    mchains = []
    for g, ranks in enumerate(GROUP_RANKS):
        glist = [None] * len(ranks)
        for r in ranks:
            pos = rank2slot[r] - base[g]
            # emission/accumulation order = chunk order (data availability)
            glist[pos] = sorted(chain_of_rank[r], key=lambda t: t[1])
        merged = []
        pos = 0
        while pos < len(glist):
            ch = glist[pos]
            if len(ch) == 1:
                px, ci, pr = ch[0]
                npair = 1
                while (pos + npair < len(glist)
                       and len(glist[pos + npair]) == 1
                       and glist[pos + npair][0][1] == ci
                       and glist[pos + npair][0][2] == pr + npair):
                    npair += 1
                merged.append((pos, [(ci, pr, npair)]))
                pos += npair
            else:
                merged.append((pos, [(ci, pr, 1) for (px, ci, pr) in ch]))
                pos += 1
        # PE emission order: chains whose data lands earliest first
        merged.sort(key=lambda m: max(it[0] for it in m[1]))
        mchains.append(merged)

    return dict(rank2slot=rank2slot, chunks=all_chunks,
                gcall_nch=gcall_nch, pairs=pair, mchains=mchains,
                base=base)


# --------------------------------------------------------------------------
# device program
# --------------------------------------------------------------------------
def _build_program(npads):
    import concourse.bacc as bacc
    import concourse.mybir as mybir
    from concourse.tile import TileContext

    plan = _make_plan(npads)
    T = len(plan["chunks"])
    PAIRS = plan["pairs"]
    gcall_nch = plan["gcall_nch"]
    base = plan["base"]

    nc = bacc.Bacc("TRN2", num_devices=N_CORES)
    dt = mybir.dt
    fcl = nc.dram_tensor("fcl", [B * H * W, C], dt.float16, kind="ExternalInput")
    amat = nc.dram_tensor("amat", [128, PAIRS, NJ], dt.float16,
                          kind="ExternalInput")
    ca = gcall_nch[0]
    pidxa = nc.dram_tensor("pidxa", [128, ca * 8], dt.int16,
                           kind="ExternalInput")
    pidxb = nc.dram_tensor("pidxb", [128, (T - ca) * 8], dt.int16,
                           kind="ExternalInput")
    outd = nc.dram_tensor("out", [128, RPC, 2, NJ], dt.float16,
                          kind="ExternalOutput")

    gc_bounds = [0]
    for n in gcall_nch:
        gc_bounds.append(gc_bounds[-1] + n)

    n_gcalls = len(gcall_nch)

    def gc_of(ci):
        for gc in range(n_gcalls):
            if ci < gc_bounds[gc + 1]:
                return gc
        raise AssertionError(ci)

    with TileContext(nc) as tc:
        with (
            tc.tile_pool(name="main", bufs=1) as mp,
            tc.tile_pool(name="psum", bufs=4, space="PSUM") as pp,
        ):
            idxa_t = mp.tile([128, ca * 8], dt.int16, tag="idxa")
            idxb_t = mp.tile([128, (T - ca) * 8], dt.int16, tag="idxb")
            a_t = mp.tile([128, PAIRS, NJ], dt.float16, tag="amat")
            ob = mp.tile([128, RPC, 2, NJ], dt.float16, tag="outbuf")
            warm = mp.tile([128, 8], dt.float16, tag="warm")
            nc.sync.dma_start(out=idxa_t[:], in_=pidxa[:])
            nc.sync.dma_start(out=idxb_t[:], in_=pidxb[:])
            nc.sync.dma_start(out=a_t[:], in_=amat[:])
            # preload the Act engine's activation table (1283ns) upfront
            nc.vector.memset(warm[:], 0.0)
            nc.scalar.activation(warm[:], warm[:],
                                 mybir.ActivationFunctionType.Copy)

            patch = []
            for gc in range(n_gcalls):
                nch = gcall_nch[gc]
                p_t = mp.tile([128, nch, C], dt.float16, tag=f"patch{gc}")
                c0 = gc_bounds[gc]
                idxs = (idxa_t[:, :] if gc == 0
                        else idxb_t[:, (c0 - ca) * 8:(c0 - ca + nch) * 8])
                nc.gpsimd.dma_gather(
                    out_ap=p_t[:],
                    in_ap=fcl[:],
                    idxs_ap=idxs,
                    num_idxs=nch * 128,
                    num_idxs_reg=nch * 128,
                    elem_size=C,
                    single_packet=False,
                )
                patch.append(p_t)

            for g, gs in enumerate(GROUP_SIZES):
                o0 = base[g]
                pbg = [pp.tile([128, gs, NJ], dt.float32, tag=f"pb{h}",
                               name=f"pb{g}_{h}") for h in range(2)]
                for (pos0, items) in plan["mchains"][g]:
                    for h in range(2):
                        for i, (ci, pr, npair) in enumerate(items):
                            gc = gc_of(ci)
                            lc = ci - gc_bounds[gc]
                            nc.tensor.matmul(
                                out=pbg[h][:, pos0:pos0 + npair, :],
                                lhsT=patch[gc][:, lc, h * 128:(h + 1) * 128],
                                rhs=a_t[:, pr:pr + npair, :],
                                start=(i == 0),
                                stop=(i == len(items) - 1),
                            )
                nc.vector.tensor_copy(out=ob[:, o0:o0 + gs, 0, :],
                                      in_=pbg[0][:])
                nc.scalar.activation(ob[:, o0:o0 + gs, 1, :], pbg[1][:],
                                     mybir.ActivationFunctionType.Copy)
                for og in OUT_GROUPS:
                    if g == og[-1]:
                        s0, s1 = base[og[0]], base[og[-1] + 1]
                        nc.sync.dma_start(out=outd[:, s0:s1, :, :],
                                          in_=ob[:, s0:s1, :, :])
    nc.compile()
    return nc, plan


# --------------------------------------------------------------------------
# entry point
# --------------------------------------------------------------------------
def kernel(input, rois, offset):
    from concourse.bass_utils import run_bass_kernel_spmd

    input = np.asarray(input, dtype=f32)
    wts = _sep_weights(rois, offset)
    npix = np.array([len(t[1]) * len(t[2]) for t in wts])

    # octet sharding: sorted desc, octet r = 8 consecutive rois -> one per
    # core, so per-rank max (npad) is tight; balance per-core totals by
    # giving the octet's biggest roi to the least-loaded core
    order = np.argsort(-npix, kind="stable")
    npads = tuple(int(max(npix[order[8 * r]], 1)) for r in range(RPC))
    core_rois = [[None] * RPC for _ in range(N_CORES)]
    totals = np.zeros(N_CORES, np.int64)
    for r in range(RPC):
        members = order[8 * r:8 * r + 8]  # desc size
        dst = np.argsort(totals, kind="stable")  # asc load
        for j, m in enumerate(members):
            core_rois[dst[j]][r] = int(m)
            totals[dst[j]] += npix[m]

    key = npads
    if key not in _prog_cache:
        _prog_cache[key] = _build_program(npads)
    nc, plan = _prog_cache[key]
    T = len(plan["chunks"])
    PAIRS = plan["pairs"]

    fcl = np.ascontiguousarray(
        input.transpose(0, 2, 3, 1).astype(np.float16)
    ).reshape(B * H * W, C)

    in_maps = []
    for k in range(N_CORES):
        logical = np.zeros(T * 128, np.int32)
        a_arr = np.zeros((128, PAIRS, NJ), np.float16)
        for ci, segs in enumerate(plan["chunks"]):
            for (r, r0, r1, px0, pr) in segs:
                roi = core_rois[k][r]
                b, ys, xs, Wy, Wx = wts[roi]
                ny, nx = len(ys), len(xs)
                np_r = ny * nx
                n = r1 - r0
                pix = px0 + np.arange(n)
                m = pix < np_r
                if not m.any():
                    continue
                pixm = pix[m]
                yy = ys[pixm // nx]
                xx = xs[pixm % nx]
                logical[ci * 128 + r0:ci * 128 + r1][m] = (
                    b * (H * W) + yy.astype(np.int32) * W + xx
                )
                a_arr[np.arange(r0, r1)[m], pr, :] = (
                    Wy[pixm // nx] * Wx[pixm % nx]
                ).astype(np.float16)
        idx16 = np.tile(logical.astype(np.int16).reshape(-1, 16).T, (8, 1))
        ca = plan["gcall_nch"][0]
        in_maps.append({"fcl": fcl, "amat": a_arr,
                        "pidxa": np.ascontiguousarray(idx16[:, :ca * 8]),
                        "pidxb": np.ascontiguousarray(idx16[:, ca * 8:])})

    res = run_bass_kernel_spmd(nc, in_maps, core_ids=list(range(N_CORES)))

    out_full = np.empty((N_ROIS, C, P, P), f32)
    for k in range(N_CORES):
        arr = res.results[k]["out"].astype(f32)  # (128, RPC, 2, 49)
        t = arr.transpose(1, 2, 0, 3).reshape(RPC, C, P, P)
        for r in range(RPC):
            out_full[core_rois[k][r]] = t[plan["rank2slot"][r]]
    return out_full
